# revision 24
# baseline (speedup 1.0000x reference)
"""Linformer self-attention on 8 Trainium2 NeuronCores.

Problem (hardcoded shapes): x [4,4096,1024] f32; per batch:
  q = scale*(x@Wq); kv = x@Wkv; keys/values compressed 4096->256 via
  proj_k/proj_v; 16-head attention (dh=64, k=256); out @ Wproj + bproj.

Sharding: 8 cores = 4 batches x 2 head-groups (8 heads / 512 cols each).
Each core computes a partial [4096,1024] output (Wproj row-split); host
sums the pair and adds bias.

Per-core dataflow (matmuls: out = lhsT.T @ rhs):
  A : xcxv[1024,512] = x.T @ [proj_k|proj_v]   fp8 DoubleRow, 3-term
      residual product (x8*kv8 + x8*kvr8 + xr8*kv8), host-split operands
  A2: kprojT[512,256] = Wk_g.T @ xc ; vproj[256,520] = xv.T @ Wv_g with
      an interleaved all-ones column per head (softmax sums come free
      out of the D matmul)
  B : qT[512,4096] = Wq_g.T @ x.T   fp8 DoubleRow 3-term; x.T sent
      pre-transposed by the host (no device transposes)
  C : per head: scores[256,512] -> exp (single fused ACT op per head)
  D : po[n,65] = pexp.T @ [v_h|1]; col 64 = sum(exp); DVE reciprocal +
      broadcast multiply normalizes into o
  E : out[n,1024] = o.T-transpose @ Wproj_g, interleaved into the next
      block's scores stream to keep the PE gapless; stores on the Pool
      SWDGE queue (bf16)
"""

import numpy as np
import ml_dtypes

import concourse.bass as bass
import concourse.mybir as mybir
import concourse.tile as tile
from concourse import bacc
from concourse.bass_utils import run_bass_kernel_spmd
from concourse.masks import make_identity

P = 128
N, D, K, DG, DH = 4096, 1024, 256, 512, 64
NB = 8                    # n-blocks of 512
F32 = mybir.dt.float32
BF16 = mybir.dt.bfloat16
F8 = mybir.dt.float8e4
F8NP = mybir.dt.np(F8)
DR = mybir.MatmulPerfMode.DoubleRow
EXP = mybir.ActivationFunctionType.Exp
MUL = mybir.AluOpType.mult

# power-of-2 pre-scales so fp8e4m3 operands sit in normal range (std ~18)
SX = 16.0                 # x ~ N(0,1)
SKV = 512.0               # proj_k/v ~ U(-1/16,1/16)
SQ = 8192.0               # scale*Wq ~ U(-1/256,1/256)
INV_A = 1.0 / (SX * SKV)
INV_B = 1.0 / (SX * SQ)

_cache = {}


def build_nc():
    nc = bacc.Bacc(None, target_bir_lowering=False, debug=False)

    x8_d = nc.dram_tensor("x8", [N, D], F8, kind="ExternalInput")
    xr8_d = nc.dram_tensor("xr8", [N, D], F8, kind="ExternalInput")
    kv8_d = nc.dram_tensor("kv8", [N, 2 * K], F8, kind="ExternalInput")
    kvr8_d = nc.dram_tensor("kvr8", [N, 2 * K], F8, kind="ExternalInput")
    xt8_d = nc.dram_tensor("xt8", [D, N], F8, kind="ExternalInput")
    xtr8_d = nc.dram_tensor("xtr8", [D, N], F8, kind="ExternalInput")
    wq8_d = nc.dram_tensor("wq8", [D, DG], F8, kind="ExternalInput")
    wqr8_d = nc.dram_tensor("wqr8", [D, DG], F8, kind="ExternalInput")
    wk_d = nc.dram_tensor("wk", [D, DG], BF16, kind="ExternalInput")
    wv_d = nc.dram_tensor("wv", [D, DG], BF16, kind="ExternalInput")
    wp_d = nc.dram_tensor("wproj", [DG, D], BF16, kind="ExternalInput")
    out_d = nc.dram_tensor("out", [N, D], BF16, kind="ExternalOutput")

    with tile.TileContext(nc) as tc:
        from contextlib import ExitStack
        with ExitStack() as ctx:
            res = ctx.enter_context(tc.tile_pool(name="res", bufs=1))
            wproj_sb = res.tile([P, 4 * D], BF16, tag="wproj")
            kprojT_sb = res.tile([P, 4 * K], BF16, tag="kprojT")
            # vproj: per fc half, 8 heads x (64 v-cols + 1 ones-col)
            vproj_sb = res.tile([P, 2 * 520], BF16, tag="vproj")
            qT_sb = res.tile([P, 4 * N], BF16, tag="qT")
            id_mm = res.tile([P, P], BF16, tag="id_mm")
            make_identity(nc, id_mm[:])
            # ones columns (col 64 of every 65-block)
            nc.vector.memset(
                vproj_sb[:].rearrange("p (h c) -> p h c", c=65)[:, :, 64:65], 1.0)

            # ---------------- input DMAs on the ACT queue ----------------
            abw = ctx.enter_context(tc.tile_pool(name="abw", bufs=1))
            wq8_sb = abw.tile([P, 8 * DG], F8, tag="wq8")
            wqr8_sb = abw.tile([P, 8 * DG], F8, tag="wqr8")
            nc.scalar.dma_start(
                out=wq8_sb[:], in_=wq8_d[:, :].rearrange("(c p) g -> p c g", p=P))
            nc.scalar.dma_start(
                out=wqr8_sb[:], in_=wqr8_d[:, :].rearrange("(c p) g -> p c g", p=P))
            xt8_sb = abw.tile([P, 8 * N], F8, tag="xt8")
            xtr8_sb = abw.tile([P, 8 * N], F8, tag="xtr8")
            wk_sb = abw.tile([P, 8 * DG], BF16, tag="wk")
            wv_sb = abw.tile([P, 8 * DG], BF16, tag="wv")
            nc.scalar.dma_start(
                out=wk_sb[:], in_=wk_d[:, :].rearrange("(c p) g -> p c g", p=P))
            nc.scalar.dma_start(
                out=wv_sb[:], in_=wv_d[:, :].rearrange("(c p) g -> p c g", p=P))
            nc.scalar.dma_start(
                out=wproj_sb[:], in_=wp_d[:, :].rearrange("(c p) d -> p c d", p=P))

            # ---------------- Phase A ----------------
            with ExitStack() as actx:
                xin = actx.enter_context(tc.tile_pool(name="xin", bufs=4))
                xcp = actx.enter_context(tc.tile_pool(name="xcp", bufs=1))
                xcxv_sb = xcp.tile([P, 8 * 2 * K], BF16, tag="xcxv")
                with tc.tile_pool(name="pa", bufs=1, space="PSUM") as pa:
                    accs = [pa.tile([P, 2 * K], F32, tag=f"pa{dd}", name=f"pa{dd}")
                            for dd in range(8)]
                    PAIR_SLABS = [1, 1, 2, 2, 2, 2, 2, 2, 2]  # 16 pairs
                    p0_ = 0
                    for si, npair in enumerate(PAIR_SLABS):
                        nch = 2 * npair
                        x8s = xin.tile([P, 4 * D], F8, tag="x8s")
                        xr8s = xin.tile([P, 4 * D], F8, tag="xr8s")
                        kv8s = xin.tile([P, 4 * 2 * K], F8, tag="kv8s")
                        kvr8s = xin.tile([P, 4 * 2 * K], F8, tag="kvr8s")
                        sl = slice(p0_ * 2 * P, (p0_ + npair) * 2 * P)
                        nc.sync.dma_start(
                            out=x8s[:, :nch * D].rearrange("p (c d) -> p c d", c=nch),
                            in_=x8_d[sl, :].rearrange("(c p) d -> p c d", p=P))
                        nc.sync.dma_start(
                            out=kv8s[:, :nch * 2 * K].rearrange("p (c d) -> p c d", c=nch),
                            in_=kv8_d[sl, :].rearrange("(c p) d -> p c d", p=P))
                        nc.sync.dma_start(
                            out=xr8s[:, :nch * D].rearrange("p (c d) -> p c d", c=nch),
                            in_=xr8_d[sl, :].rearrange("(c p) d -> p c d", p=P))
                        nc.sync.dma_start(
                            out=kvr8s[:, :nch * 2 * K].rearrange("p (c d) -> p c d", c=nch),
                            in_=kvr8_d[sl, :].rearrange("(c p) d -> p c d", p=P))
                        if si == 3:
                            # gate the (large, late-needed) xT loads on slab 2
                            # having arrived: WAW with these dummy writes
                            nc.vector.tensor_copy(
                                xt8_sb[:].rearrange("p (c n) -> p c n", c=8)[:, :, 0:1],
                                x8s[:].rearrange("p (c d) -> p c d", c=8)[:, :, 0:1])
                            nc.vector.tensor_copy(
                                xtr8_sb[:].rearrange("p (c n) -> p c n", c=8)[:, :, 0:1],
                                x8s[:].rearrange("p (c d) -> p c d", c=8)[:, :, 0:1])
                            for dd in range(8):
                                nc.scalar.dma_start(
                                    out=xt8_sb[:, dd * N:(dd + 1) * N],
                                    in_=xt8_d[dd * P:(dd + 1) * P, :])
                                nc.scalar.dma_start(
                                    out=xtr8_sb[:, dd * N:(dd + 1) * N],
                                    in_=xtr8_d[dd * P:(dd + 1) * P, :])
                        x8v = x8s[:].rearrange("p (c d) -> p c d", c=4)
                        xr8v = xr8s[:].rearrange("p (c d) -> p c d", c=4)
                        kv8v = kv8s[:].rearrange("p (c d) -> p c d", c=4)
                        kvr8v = kvr8s[:].rearrange("p (c d) -> p c d", c=4)
                        last = si == len(PAIR_SLABS) - 1
                        terms = ((x8v, kv8v), (x8v, kvr8v), (xr8v, kv8v))
                        if last:
                            # dd-major: each acc finishes early so evictions
                            # overlap the remaining matmuls
                            for dd in range(8):
                                dsl = slice(dd * P, (dd + 1) * P)
                                for ti, (lv, rv) in enumerate(terms):
                                    for t in range(npair):
                                        cp = slice(2 * t, 2 * t + 2)
                                        nc.tensor.matmul(
                                            accs[dd][:],
                                            lhsT=lv[:, cp, dsl],
                                            rhs=rv[:, cp, :],
                                            start=False,
                                            stop=(ti == 2 and t == npair - 1),
                                            perf_mode=DR)
                        else:
                            for ti, (lv, rv) in enumerate(terms):
                                for t in range(npair):
                                    cp = slice(2 * t, 2 * t + 2)
                                    for dd in range(8):
                                        dsl = slice(dd * P, (dd + 1) * P)
                                        nc.tensor.matmul(
                                            accs[dd][:],
                                            lhsT=lv[:, cp, dsl],
                                            rhs=rv[:, cp, :],
                                            start=(si == 0 and ti == 0 and t == 0),
                                            stop=False,
                                            perf_mode=DR)
                        p0_ += npair
                    for dd in range(8):
                        dst = xcxv_sb[:, dd * 2 * K:(dd + 1) * 2 * K]
                        if dd % 2 == 0:
                            nc.scalar.mul(dst, accs[dd][:], INV_A)
                        else:
                            nc.vector.tensor_scalar_mul(dst, accs[dd][:], INV_A)

                # ---------------- Phase A2 ----------------
                with tc.tile_pool(name="pa2", bufs=1, space="PSUM") as pa2:
                    kps = [pa2.tile([P, K], F32, tag=f"kp{jc}", name=f"kp{jc}")
                           for jc in range(4)]
                    vps = [pa2.tile([P, DG], F32, tag=f"vp{fc}", name=f"vp{fc}")
                           for fc in range(2)]
                    for dd in range(8):
                        for jc in range(4):
                            nc.tensor.matmul(
                                kps[jc][:],
                                lhsT=wk_sb[:, dd * DG + jc * P: dd * DG + (jc + 1) * P],
                                rhs=xcxv_sb[:, dd * 2 * K: dd * 2 * K + K],
                                start=(dd == 0), stop=(dd == 7))
                        for fc in range(2):
                            nc.tensor.matmul(
                                vps[fc][:],
                                lhsT=xcxv_sb[:, dd * 2 * K + K + fc * P:
                                             dd * 2 * K + K + (fc + 1) * P],
                                rhs=wv_sb[:, dd * DG:(dd + 1) * DG],
                                start=(dd == 0), stop=(dd == 7))
                    for jc in range(4):
                        nc.scalar.copy(out=kprojT_sb[:, jc * K:(jc + 1) * K],
                                       in_=kps[jc][:])
                    for fc in range(2):
                        nc.vector.tensor_copy(
                            vproj_sb[:, fc * 520:(fc + 1) * 520]
                            .rearrange("p (h c) -> p h c", c=65)[:, :, 0:64],
                            vps[fc][:].rearrange("p (h c) -> p h c", c=64))

            # ---------------- Phase B: qT ----------------
            pex = ctx.enter_context(tc.tile_pool(name="pex", bufs=8))
            warm_pexps = []

            def emit_scores(nbs, h, atpool, tag):
                jc, p0 = h // 2, (h % 2) * DH
                at = atpool.tile([P, 2 * DG], F32, tag=tag, bufs=2, name="at")
                for fc in range(2):
                    nc.tensor.matmul(
                        at[:, fc * DG:(fc + 1) * DG],
                        lhsT=kprojT_sb[p0:p0 + DH,
                                       jc * K + fc * P: jc * K + (fc + 1) * P],
                        rhs=qT_sb[p0:p0 + DH,
                                  jc * N + nbs * DG: jc * N + (nbs + 1) * DG],
                        start=True, stop=True)
                pexp = pex.tile([P, 2 * DG], BF16, tag="pexp", name="pexp")
                nc.scalar.activation(pexp[:], at[:], EXP)
                return pexp

            with ExitStack() as bctx:
                pq = bctx.enter_context(tc.tile_pool(name="pq", bufs=3, space="PSUM"))
                warm = bctx.enter_context(tc.tile_pool(name="warm", bufs=1, space="PSUM"))
                wq8v = wq8_sb[:].rearrange("p (c g) -> p c g", c=8)
                wqr8v = wqr8_sb[:].rearrange("p (c g) -> p c g", c=8)
                xt8v = xt8_sb[:].rearrange("p (c n) -> p c n", c=8)
                xtr8v = xtr8_sb[:].rearrange("p (c n) -> p c n", c=8)
                for nb in range(NB):
                    nsl = slice(nb * DG, (nb + 1) * DG)
                    for jc in range(4):
                        acc = pq.tile([P, DG], F32, tag="pq")
                        gsl = slice(jc * P, (jc + 1) * P)
                        for t in range(4):
                            cp = slice(2 * t, 2 * t + 2)
                            for ti, (lv, rv) in enumerate(
                                    ((wq8v, xt8v), (wq8v, xtr8v), (wqr8v, xt8v))):
                                nc.tensor.matmul(
                                    acc[:],
                                    lhsT=lv[:, cp, gsl],
                                    rhs=rv[:, cp, nsl],
                                    start=(t == 0 and ti == 0),
                                    stop=(t == 3 and ti == 2),
                                    perf_mode=DR)
                        dst = qT_sb[:, jc * N + nb * DG: jc * N + (nb + 1) * DG]
                        if jc % 2 == 0:
                            nc.vector.tensor_scalar_mul(dst, acc[:], INV_B)
                        else:
                            nc.scalar.mul(dst, acc[:], INV_B)
                    # warm-start block 0 of phase C while B still owns the PE
                    if 2 <= nb < 6:
                        for wh in ((nb - 2) * 2, (nb - 2) * 2 + 1):
                            warm_pexps.append(emit_scores(0, wh, warm, "wat"))

            # ---------------- Phase C/D/E fused per n-block ----------------
            # per nb: scores/exp for 8 heads with the PREVIOUS block's phase E
            # interleaved between heads (fills PE while ACT runs exp), then
            # phase D + normalization, then transposes (SP DMas run during the
            # next block's scores).
            with ExitStack() as cctx:
                osb = cctx.enter_context(tc.tile_pool(name="osb", bufs=2))
                otp = cctx.enter_context(tc.tile_pool(name="otp", bufs=2))
                outp = cctx.enter_context(tc.tile_pool(name="outp", bufs=3))
                rcp = cctx.enter_context(tc.tile_pool(name="rcp", bufs=3))
                pc = cctx.enter_context(tc.tile_pool(name="pc", bufs=1, space="PSUM"))

                prev = None  # (nb, [ot x4]) pending phase E

                def emit_E_part(nn2, half, ots, outsb, nbp, evict_eng=None):
                    pe = pc.tile([P, DG], F32, tag="work", bufs=4, name="pe")
                    for jc2 in range(4):
                        nc.tensor.matmul(
                            pe[:],
                            lhsT=ots[nn2][:, jc2 * P:(jc2 + 1) * P],
                            rhs=wproj_sb[:, jc2 * D + half * DG: jc2 * D + (half + 1) * DG],
                            start=(jc2 == 0), stop=(jc2 == 3))
                    eng = evict_eng or nc.vector
                    dst = outsb[:, half * DG:(half + 1) * DG]
                    if eng is nc.scalar:
                        eng.copy(out=dst, in_=pe[:])
                    else:
                        eng.tensor_copy(dst, pe[:])
                    if half == 1:
                        ci = nbp * 4 + nn2
                        nc.gpsimd.dma_start(out=out_d[ci * P:(ci + 1) * P, :],
                                            in_=outsb[:])

                for nb in range(NB):
                    outsb_cur = None
                    if nb == 0:
                        pexps = warm_pexps
                    else:
                        pexps = []
                        for h in range(8):
                            pexps.append(emit_scores(nb, h, pc, "at"))
                            # interleave previous block's phase E
                            if prev is not None:
                                nn2, half = h // 2, h % 2
                                if half == 0:
                                    outsb_cur = outp.tile([P, D], BF16, tag="outsb",
                                                          name="outsb")
                                emit_E_part(nn2, half, prev[1], outsb_cur, prev[0],
                                            evict_eng=nc.scalar if half == 0
                                            else nc.vector)
                    # phase D + normalization
                    o_big = osb.tile([P, 4 * DG], BF16, tag="obig")
                    o_bigv = o_big[:].rearrange("p (n g) -> p n g", n=4)
                    for h in range(8):
                        pexp = pexps[h]
                        po = pc.tile([P, DG], F32, tag="work", bufs=4, name="po")
                        pov = po[:, :260].rearrange("p (n c) -> p n c", c=65)
                        for nn2 in range(4):
                            for fc in range(2):
                                nc.tensor.matmul(
                                    po[:, nn2 * 65:(nn2 + 1) * 65],
                                    lhsT=pexp[:, fc * DG + nn2 * P: fc * DG + (nn2 + 1) * P],
                                    rhs=vproj_sb[:, fc * 520 + h * 65: fc * 520 + (h + 1) * 65],
                                    start=(fc == 0), stop=(fc == 1))
                        rec = rcp.tile([P, 4], F32, tag="rec", name="rec")
                        rec3 = rec[:].rearrange("p (n o) -> p n o", o=1)
                        nc.vector.reciprocal(rec3, pov[:, :, 64:65])
                        in0 = pov[:, :, 0:64]
                        out_ap = o_bigv[:, :, h * DH:(h + 1) * DH]
                        if nb == NB - 1 and h % 2 == 1:
                            # last block: split the normalize chain onto ACT so
                            # the drain's transposes aren't gated on one engine
                            for nn2 in range(4):
                                nc.scalar.mul(out_ap[:, nn2:nn2 + 1, :]
                                              .rearrange("p a c -> p (a c)"),
                                              pov[:, nn2:nn2 + 1, 0:64]
                                              .rearrange("p a c -> p (a c)"),
                                              rec[:, nn2:nn2 + 1])
                        else:
                            in1, _ = bass.broadcast_tensor_aps(rec3, in0)
                            nc.vector.tensor_tensor(out_ap, in0, in1, MUL)
                    # PE-transpose o into lhsT layout for phase E (jc2-major
                    # so early transposes only wait on the first heads' muls)
                    otps = [pc.tile([P, DG], BF16, tag="work", bufs=4,
                                    name=f"otps{i}") for i in range(4)]
                    for jc2 in range(4):
                        for nn2 in range(4):
                            nc.tensor.transpose(
                                otps[nn2][:, jc2 * P:(jc2 + 1) * P],
                                o_big[:, nn2 * DG + jc2 * P: nn2 * DG + (jc2 + 1) * P],
                                id_mm[:])
                    ots = [otp.tile([P, DG], BF16, tag=f"ot{i}", name=f"ot{i}")
                           for i in range(4)]
                    for nn2 in range(4):
                        nc.vector.tensor_copy(ots[nn2][:], otps[nn2][:])
                    prev = (nb, ots)

                # drain: phase E for the last n-block
                engs = [nc.vector, nc.scalar]
                for nn2 in range(4):
                    outsb_cur = outp.tile([P, D], BF16, tag="outsb", name="outsb")
                    for half in range(2):
                        emit_E_part(nn2, half, prev[1], outsb_cur, prev[0],
                                    evict_eng=engs[(nn2 + half) % 2])
    nc.compile()
    return nc


def _split8(a, s):
    a = np.asarray(a, np.float32) * np.float32(s)
    h = a.astype(F8NP)
    r = (a - h.astype(np.float32)).astype(F8NP)
    return h, r


def kernel(x, Wq, Wkv, Wproj, bproj, proj_k, proj_v):
    import os
    x = np.asarray(x, np.float32)
    Wq, Wkv, Wproj = (np.asarray(a, np.float32) for a in (Wq, Wkv, Wproj))
    bproj = np.asarray(bproj, np.float32)
    proj_k, proj_v = np.asarray(proj_k, np.float32), np.asarray(proj_v, np.float32)

    if "nc" not in _cache:
        _cache["nc"] = build_nc()
    nc = _cache["nc"]

    scale = np.float32(DH ** -0.5)
    kv8, kvr8 = _split8(np.concatenate([proj_k, proj_v], axis=1), SKV)
    xs, xts = [], []
    for b in range(4):
        x8, xr8 = _split8(x[b], SX)
        xs.append((x8, xr8))
        xts.append((np.ascontiguousarray(x8.T), np.ascontiguousarray(xr8.T)))
    bf = lambda a: np.ascontiguousarray(a, ml_dtypes.bfloat16)
    in_maps = []
    for c in range(8):
        b, g = c // 2, c % 2
        cols = slice(g * DG, (g + 1) * DG)
        wq8, wqr8 = _split8(scale * Wq[:, cols], SQ)
        in_maps.append({
            "x8": xs[b][0], "xr8": xs[b][1],
            "kv8": kv8, "kvr8": kvr8,
            "xt8": xts[b][0], "xtr8": xts[b][1],
            "wq8": wq8, "wqr8": wqr8,
            "wk": bf(Wkv[:, :D][:, cols]),
            "wv": bf(Wkv[:, D:][:, cols]),
            "wproj": bf(Wproj[cols, :]),
        })
    res = run_bass_kernel_spmd(nc, in_maps, list(range(8)),
                               trace=bool(os.environ.get("LINF_TRACE")))
    _cache["last_result"] = res
    _cache["in_maps"] = in_maps
    outs = [r["out"] for r in res.results]
    full = np.stack([outs[2 * b].astype(np.float32) + outs[2 * b + 1].astype(np.float32)
                     for b in range(4)])
    return (full + bproj).astype(np.float32)
